# revision 1
# baseline (speedup 1.0000x reference)
"""Trainium2 Bass kernel for nn_ColorHistograms.

Pipeline (per NeuronCore, 2 batch elements each, 8 cores):
  1. Stream x tiles [128 frames, 3888] from HBM in 3 chunk-DMAs each (spreads
     load over DMA engines; one engine tops out at ~22.5 GB/s). Per-channel
     spatial means: channels 0,1 on ScalarE (activation Copy + accum_out),
     channel 2 on VectorE (strided tensor_reduce).
  2. PE-transpose the [128, 24] mean columns; stage them to a DRAM scratch as
     fp16 centered at 0.5 (mean-of-1296-uniforms is 0.5 +- 0.01, and only
     differences matter downstream, so fp16 keeps ~1e-3 relative accuracy
     while halving gather traffic).
  3. Toeplitz gather DMAs (one per channel) materialize all 101 shifted
     copies of the padded mean rows; a 0-stride gather broadcasts the base.
  4. VectorE: fp16 in-place diff, |.| channel-reduce to fp32, out-of-range
     mask multiply -> window features [101, 1024] (+ ones row = bias trick).
  5. PE matmul [102]x[128 t]x[128 out] with fc weights (bias folded in as an
     extra contraction row), VectorE relu PSUM->SBUF, contiguous DMA out.
"""

import sys

if "/opt/trn_rl_repo" not in sys.path:
    sys.path.insert(0, "/opt/trn_rl_repo")

import numpy as np

N_CORES = 8
B, T, H, W, C = 16, 1024, 27, 48, 3
S = H * W                 # 1296 spatial positions
ROW = S * C               # 3888 floats per frame
LW = 101                  # lookup window
PAD = 50
OD = 128                  # output dim
BPC = B // N_CORES        # batches per core = 2
PADROW = T + LW - 1       # 1124
FT = T // 128             # 8 frame-tiles per batch
XCH = 2                   # chunk-DMAs per x tile
CENTER = 0.5              # mean centering before the fp16 staging cast
# The runtime picks the DMA engine from the DRAM-side address granule, so a
# gather whose reads all land in one ~13 KB region serializes onto a single
# engine (~24 GB/s). Stage the mean rows into NREP replicas spaced REPS
# elements apart and split each gather into w-chunks reading distinct
# replicas so the chunks land on distinct engines. The stride is an odd
# multiple of 2/4/8 KB so replicas stay distinct mod 16 for any granule size.
NREP = 4
REPS = 70656              # fp16 elements between replicas (141312 B)
# window rows are PERMUTED so that dest row 0 is w=50 (the base row): the
# on-chip base broadcast can then read partition 0 of the gather tile with no
# extra row DMA. fc weights and the mask are row-permuted on the host to
# match, so the matmul contraction is unchanged.
PERM = [50] + [w for w in range(LW) if w != 50]
# (dest_row0, dest_row1, src_w0, replica)
WCHUNKS = [(0, 1, 50, 3), (1, 26, 0, 0), (26, 51, 25, 1),
           (51, 76, 51, 2), (76, 101, 76, 3)]

_CACHE = {}


def _build_program():
    import concourse.bass as bass
    import concourse.tile as tile
    from concourse import bacc, mybir
    from concourse.ap import AP

    f32 = mybir.dt.float32
    f16 = mybir.dt.float16
    bf16 = mybir.dt.bfloat16
    nc = bacc.Bacc("TRN2", target_bir_lowering=False, debug=False)

    xs = nc.dram_tensor("xs", [BPC * T, ROW], f32, kind="ExternalInput")
    fcwb = nc.dram_tensor("fcwb", [LW + 1, OD], f32, kind="ExternalInput")
    maskw = nc.dram_tensor("maskw", [LW, T], bf16, kind="ExternalInput")
    ident = nc.dram_tensor("ident", [128, 128], f32, kind="ExternalInput")
    y = nc.dram_tensor("y", [BPC * T, OD], f32, kind="ExternalOutput")
    mcpad = nc.dram_tensor("mcpad", [NREP * REPS], f16)
    mc_ap = mcpad[:]

    def mc_view(offset, dims):
        return AP(tensor=mc_ap.tensor, offset=offset, ap=tuple(dims))

    with tile.TileContext(nc) as tc:
        with (
            tc.tile_pool(name="consts", bufs=1) as consts,
            tc.tile_pool(name="xin", bufs=9) as xin,
            tc.tile_pool(name="junk", bufs=1) as junkp,
            tc.tile_pool(name="sums", bufs=2) as sumsp,
            tc.tile_pool(name="stg", bufs=2) as stgp,
            tc.tile_pool(name="gath", bufs=2) as gathp,
            tc.tile_pool(name="wf", bufs=2) as wfp,
            tc.tile_pool(name="outs", bufs=4) as outsp,
            tc.tile_pool(name="zrow", bufs=1) as zrowp,
            tc.tile_pool(name="pst", bufs=2, space="PSUM") as pst,
            tc.tile_pool(name="pso", bufs=4, space="PSUM") as pso,
        ):
            fcwb_sb = consts.tile([LW + 1, OD], f32)
            nc.gpsimd.dma_start(fcwb_sb[:], fcwb[:])
            maskw_sb = consts.tile([LW, T], bf16)
            nc.gpsimd.dma_start(maskw_sb[:], maskw[:])
            ident_sb = consts.tile([128, 128], f32)
            nc.gpsimd.dma_start(ident_sb[:], ident[:])

            # zero-fill the used region of each replica (the padded wings
            # must read as 0.0; the inter-replica gaps are never read)
            z = zrowp.tile([24, PADROW], f16)
            nc.vector.memset(z[:], 0.0)
            nc.gpsimd.dma_start(
                mc_view(0, [(REPS, NREP), (1, BPC * C * PADROW)]), z[:]
            )

            # all x-chunk DMAs are emitted FIRST (no input deps) and split
            # across BOTH HWDGE rings (sync + scalar): each ring's ~1.8us
            # per-instruction descriptor-gen is serial, so two rings halve
            # the issue ramp. Emitting them before any compute keeps every
            # ring's FIFO free of wait-inversions.
            xts = []
            for b in range(BPC):
                for i in range(FT):
                    xt = xin.tile([128, ROW], f32)
                    xts.append(xt)
                    for k in range(XCH):
                        lo, hi = k * (ROW // XCH), (k + 1) * (ROW // XCH)
                        eng = nc.sync if k % 2 == 0 else nc.scalar
                        eng.dma_start(
                            xt[:, lo:hi],
                            xs[b * T + i * 128 : b * T + (i + 1) * 128, lo:hi],
                        )

            # -------- emission helpers (order = per-engine priority) --------
            sums_t, stg_t, sh_t, ba_t, wf_t = {}, {}, {}, {}, {}

            def stageA_tile(b, i):
                # channel 0 on ScalarE (activation accum), channels 1 and 2
                # on VectorE (strided reduce) - balances the two engines
                # under the ~96us stream time
                sums = sums_t[b]
                xv = xts[b * FT + i][:].rearrange("p (s c) -> p c s", c=C)
                junk = junkp.tile([128, S], f32)
                nc.scalar.activation(
                    junk[:],
                    xv[:, 0, :],
                    mybir.ActivationFunctionType.Copy,
                    bias=0.0,
                    scale=1.0 / S,
                    accum_out=sums[:, i : i + 1],
                )
                for c in (1, 2):
                    nc.vector.tensor_reduce(
                        sums[:, c * FT + i : c * FT + i + 1],
                        xv[:, c, :],
                        axis=mybir.AxisListType.X,
                        op=mybir.AluOpType.add,
                    )

            def stageA_finish(b):
                # normalize the raw VectorE sums, transpose [128, 24] ->
                # [24, 128], center+cast to fp16, stage to the DRAM replicas
                sums = sums_t[b]
                nc.vector.tensor_scalar_mul(
                    sums[:, FT : C * FT], sums[:, FT : C * FT], 1.0 / S
                )
                ps = pst.tile([C * FT, 128], f32)
                nc.tensor.transpose(ps[:], sums[:], ident_sb[:])
                stg = stgp.tile([C * FT, 128], f16)
                nc.vector.tensor_scalar_sub(stg[:], ps[:], CENTER)
                for r in range(NREP):
                    nc.gpsimd.dma_start(
                        mc_view(
                            r * REPS + b * C * PADROW + PAD,
                            [(PADROW, C), (128, FT), (1, 128)],
                        ),
                        stg[:],
                    )

            def stageB_gather(b):
                # gathers on the scalar HWDGE ring (sync carries the stream);
                # base row (= permuted row 0 of sh) replicated on gpsimd
                sh = gathp.tile([LW, C * T], f16, tag="sh")
                ba = gathp.tile([LW, C * T], f16, tag="ba")
                sh_t[b], ba_t[b] = sh, ba
                for r0, r1, w0, rep in WCHUNKS:
                    nc.scalar.dma_start(
                        sh[r0:r1, :],
                        mc_view(
                            rep * REPS + b * C * PADROW + w0,
                            [(1, r1 - r0), (PADROW, C), (1, T)],
                        ),
                    )
                nc.gpsimd.partition_broadcast(ba[:], sh[0:1, :], channels=LW)

            def stageB_dist(b, sub_eng):
                # |sh - ba| channel-reduce + out-of-range mask -> wf.
                # For batch 0 the subtract runs on gpsimd so VectorE (busy
                # pacing batch 1 tile reduces mid-stream) never stalls.
                sh, ba = sh_t[b], ba_t[b]
                wf = wfp.tile([LW + 1, T], f32)
                wf_t[b] = wf
                # row LW must be 1.0 (bias trick); engines can only start at
                # partition 0/32/64/96: fill everything, overwrite rows 0..100
                sub_eng.memset(wf[:], 1.0)
                sub_eng.tensor_sub(sh[:], sh[:], ba[:])
                shv = sh[:].rearrange("p (c t) -> p t c", c=C)
                nc.vector.tensor_reduce(
                    wf[0:LW, :],
                    shv,
                    axis=mybir.AxisListType.X,
                    op=mybir.AluOpType.add,
                    apply_absolute_value=True,
                )
                nc.vector.tensor_mul(wf[0:LW, :], wf[0:LW, :], maskw_sb[:])

            def stageC(b):
                wf = wf_t[b]
                for j in range(FT):
                    po = pso.tile([128, OD], f32)
                    nc.tensor.matmul(po[:], wf[:, bass.ts(j, 128)], fcwb_sb[:])
                    osb = outsp.tile([128, OD], f32)
                    nc.vector.tensor_scalar_max(osb[:], po[:], 0.0)
                    # batch 0 outputs go mid-stream on the idle gpsimd ring;
                    # batch 1 outputs are on the tail - split across the
                    # sync+scalar rings (idle by then) to halve issue time
                    if b == 0:
                        eng = nc.gpsimd
                    else:
                        eng = nc.sync if j % 2 == 0 else nc.scalar
                    eng.dma_start(
                        y[b * T + j * 128 : b * T + (j + 1) * 128, :], osb[:]
                    )

            # -------- emission schedule --------
            # b0's stage B/C is woven between b1's tile reduces so its
            # windowed distances + matmuls overlap the second half of the
            # x-stream without stalling any engine's FIFO.
            for b in range(BPC):
                sums_t[b] = sumsp.tile(
                    [128, C * FT], f32, name="sums", tag="sums"
                )
            for i in range(FT):
                stageA_tile(0, i)
            stageA_finish(0)
            stageB_gather(0)
            for i in range(0, 4):
                stageA_tile(1, i)
            stageB_dist(0, nc.gpsimd)
            stageC(0)
            for i in range(4, FT):
                stageA_tile(1, i)
            stageA_finish(1)
            stageB_gather(1)
            stageB_dist(1, nc.vector)
            stageC(1)

    nc.compile()
    return nc


def get_nc():
    if "nc" not in _CACHE:
        _CACHE["nc"] = _build_program()
    return _CACHE["nc"]


def make_host_inputs(x, fc_w, fc_b):
    """Per-core input maps from the full problem inputs."""
    x = np.ascontiguousarray(x, dtype=np.float32).reshape(B, T, ROW)
    wT = fc_w.T.astype(np.float32)[PERM]          # window-row permutation
    fcwb = np.concatenate([wT, fc_b[None, :].astype(np.float32)], axis=0)
    fcwb = np.ascontiguousarray(fcwb)
    u = np.arange(T)[None, :] + np.arange(LW)[:, None] - PAD
    import ml_dtypes

    maskw = ((u >= 0) & (u < T)).astype(ml_dtypes.bfloat16)[PERM]
    maskw = np.ascontiguousarray(maskw)
    ident = np.eye(128, dtype=np.float32)
    in_maps = []
    for ci in range(N_CORES):
        shard = np.ascontiguousarray(
            x[ci * BPC : (ci + 1) * BPC].reshape(BPC * T, ROW)
        )
        in_maps.append(
            {"xs": shard, "fcwb": fcwb, "maskw": maskw, "ident": ident}
        )
    return in_maps


def kernel(x, fc_w, fc_b):
    from concourse.bass_utils import run_bass_kernel_spmd

    nc = get_nc()
    in_maps = make_host_inputs(x, fc_w, fc_b)
    res = run_bass_kernel_spmd(nc, in_maps, list(range(N_CORES)))
    outs = [r["y"].reshape(BPC, T, OD) for r in res.results]
    return np.concatenate(outs, axis=0).astype(np.float32)



# revision 2
# speedup vs baseline: 1.2529x; 1.2529x over previous
"""Trainium2 Bass kernel for nn_ColorHistograms.

Pipeline (per NeuronCore, 2 batch elements each, 8 cores):
  1. Stream x tiles [128 frames, 3888] from HBM, one 2MB DMA per tile, ALL on
     the sync HWDGE ring (the scalar ring is reserved for latency-critical
     small DMAs so they never queue behind the stream). Per-tile spatial
     sums: channel 0 on ScalarE (activation Copy + accum_out, raw), channels
     1,2 on VectorE (one strided tensor_reduce, raw).
  2. PE-transpose the [128, 24] raw sum columns to PSUM; one ScalarE
     activation applies scale=1/S (mean) and bias=-0.5 (centering) and casts
     to fp16; stage to a DRAM scratch in NREP replicas (mean-of-1296-uniforms
     is 0.5 +- 0.01 and only differences matter downstream, so fp16 keeps
     ~1e-3 relative accuracy while halving gather traffic).
  3. Toeplitz gather DMAs (w-chunks reading distinct replicas so the runtime
     assigns distinct SDMA engines) materialize all 101 shifted copies of the
     padded mean rows on the scalar ring (+ sync ring for batch 1's tail).
  4. Base-row broadcast via a rank-1 PE matmul (ones[1,101]^T @ sh[0:1]) into
     PSUM; VectorE subtracts it from sh in t-halves, |.| channel-reduce to
     fp32 window features [101, 1024] (+ ones row = bias trick). Only tiles
     0 and 7 need the out-of-range mask multiply.
  5. PE matmul [102]x[128 t]x[128 out] with fc weights (bias folded in as an
     extra contraction row), VectorE relu PSUM->SBUF, contiguous DMA out.
"""

import sys

if "/opt/trn_rl_repo" not in sys.path:
    sys.path.insert(0, "/opt/trn_rl_repo")

import numpy as np

N_CORES = 8
B, T, H, W, C = 16, 1024, 27, 48, 3
S = H * W                 # 1296 spatial positions
ROW = S * C               # 3888 floats per frame
LW = 101                  # lookup window
PAD = 50
OD = 128                  # output dim
BPC = B // N_CORES        # batches per core = 2
PADROW = T + LW - 1       # 1124
FT = T // 128             # 8 frame-tiles per batch
TH = T // 2               # 512, the t-half used to pipeline the tail
CENTER = 0.5              # mean centering applied in the fp16 staging cast
# The runtime picks the DMA engine from the DRAM-side address granule, so a
# gather whose reads all land in one ~13 KB region serializes onto a single
# engine (~24 GB/s). Stage the mean rows into NREP replicas spaced REPS
# elements apart and split each gather into w-chunks reading distinct
# replicas so the chunks land on distinct engines. The stride is an odd
# multiple of 2/4/8 KB so replicas stay distinct mod 16 for any granule size.
NREP = 4
REPS = 70656              # fp16 elements between replicas (141312 B)
# window rows are PERMUTED so that dest row 0 is w=50 (the base row): the
# rank-1 PE broadcast can then read partition 0 of the gather tile. fc
# weights and the mask are row-permuted on the host to match, so the matmul
# contraction is unchanged.
PERM = [50] + [w for w in range(LW) if w != 50]
# (dest_row0, dest_row1, src_w0, replica)
WCHUNKS = [(0, 1, 50, 3), (1, 26, 0, 0), (26, 51, 25, 1),
           (51, 76, 51, 2), (76, 101, 76, 3)]

_CACHE = {}


def _build_program():
    import concourse.bass as bass
    import concourse.tile as tile
    from concourse import bacc, mybir
    from concourse.ap import AP

    f32 = mybir.dt.float32
    f16 = mybir.dt.float16
    bf16 = mybir.dt.bfloat16
    nc = bacc.Bacc("TRN2", target_bir_lowering=False, debug=False)

    xs = nc.dram_tensor("xs", [BPC * T, ROW], f32, kind="ExternalInput")
    fcwb = nc.dram_tensor("fcwb", [LW + 1, OD], f32, kind="ExternalInput")
    maskw = nc.dram_tensor("maskw", [LW, T], bf16, kind="ExternalInput")
    ident = nc.dram_tensor("ident", [128, 128], f32, kind="ExternalInput")
    y = nc.dram_tensor("y", [BPC * T, OD], f32, kind="ExternalOutput")
    mcpad = nc.dram_tensor("mcpad", [NREP * REPS], f16)
    mc_ap = mcpad[:]

    def mc_view(offset, dims):
        return AP(tensor=mc_ap.tensor, offset=offset, ap=tuple(dims))

    with tile.TileContext(nc) as tc:
        with (
            tc.tile_pool(name="consts", bufs=1) as consts,
            tc.tile_pool(name="xin", bufs=9) as xin,
            tc.tile_pool(name="junk", bufs=1) as junkp,
            tc.tile_pool(name="sums", bufs=2) as sumsp,
            tc.tile_pool(name="stg", bufs=2) as stgp,
            tc.tile_pool(name="gath", bufs=2) as gathp,
            tc.tile_pool(name="wf", bufs=2) as wfp,
            tc.tile_pool(name="outs", bufs=4) as outsp,
            tc.tile_pool(name="zrow", bufs=1) as zrowp,
            tc.tile_pool(name="pst", bufs=1, space="PSUM") as pst,
            tc.tile_pool(name="pba", bufs=1, space="PSUM") as pbap,
            tc.tile_pool(name="pso", bufs=4, space="PSUM") as pso,
        ):
            fcwb_sb = consts.tile([LW + 1, OD], f32)
            nc.gpsimd.dma_start(fcwb_sb[:], fcwb[:])
            maskw_sb = consts.tile([LW, T], bf16)
            nc.gpsimd.dma_start(maskw_sb[:], maskw[:])
            ident_sb = consts.tile([128, 128], f32)
            nc.gpsimd.dma_start(ident_sb[:], ident[:])
            ones_sb = consts.tile([1, LW], f16)
            nc.gpsimd.memset(ones_sb[:], 1.0)

            # zero-fill the used region of each replica (the padded wings
            # must read as 0.0; the inter-replica gaps are never read)
            z = zrowp.tile([24, PADROW], f16)
            nc.gpsimd.memset(z[:], 0.0)
            nc.gpsimd.dma_start(
                mc_view(0, [(REPS, NREP), (1, BPC * C * PADROW)]), z[:]
            )

            # the full x stream on the sync HWDGE ring, one DMA per tile;
            # emitted first so the ring FIFO is pure-stream and in order
            xts = []
            for b in range(BPC):
                for i in range(FT):
                    xt = xin.tile([128, ROW], f32)
                    xts.append(xt)
                    nc.sync.dma_start(
                        xt[:],
                        xs[b * T + i * 128 : b * T + (i + 1) * 128, :],
                    )

            # -------- emission helpers (order = per-engine priority) --------
            sums_t, sh_t, wf_t = {}, {}, {}

            def stageA_tile(b, i):
                # channel 0 on ScalarE (activation accum, raw), channels 1,2
                # in one strided VectorE reduce (raw); the scale 1/S and
                # center shift are applied once post-transpose on ScalarE
                sums = sums_t[b]
                xv = xts[b * FT + i][:].rearrange("p (s c) -> p c s", c=C)
                junk = junkp.tile([128, S], f32)
                nc.scalar.activation(
                    junk[:],
                    xv[:, 0, :],
                    mybir.ActivationFunctionType.Copy,
                    bias=0.0,
                    scale=1.0,
                    accum_out=sums[:, i * C : i * C + 1],
                )
                nc.vector.tensor_reduce(
                    sums[:, i * C + 1 : i * C + 3],
                    xv[:, 1:3, :],
                    axis=mybir.AxisListType.X,
                    op=mybir.AluOpType.add,
                )

            def stageA_finish(b):
                # transpose raw sums [128, 24] -> [24, 128] on PE, then one
                # ScalarE activation does mean-scale + center + fp16 cast,
                # then stage to the DRAM replicas (SWDGE, casts not needed)
                sums = sums_t[b]
                ps = pst.tile([C * FT, 128], f32)
                nc.tensor.transpose(ps[:], sums[:], ident_sb[:])
                stg = stgp.tile([C * FT, 128], f16)
                nc.scalar.activation(
                    stg[:],
                    ps[:],
                    mybir.ActivationFunctionType.Copy,
                    bias=-CENTER,
                    scale=1.0 / S,
                )
                for r in range(NREP):
                    nc.gpsimd.dma_start(
                        mc_view(
                            r * REPS + b * C * PADROW + PAD,
                            [(128, FT), (PADROW, C), (1, 128)],
                        ),
                        stg[:],
                    )

            def stageB_gather(b, engs):
                sh = gathp.tile([LW, C * T], f16, tag="sh")
                sh_t[b] = sh
                for ci, (r0, r1, w0, rep) in enumerate(WCHUNKS):
                    engs[ci % len(engs)].dma_start(
                        sh[r0:r1, :],
                        mc_view(
                            rep * REPS + b * C * PADROW + w0,
                            [(1, r1 - r0), (PADROW, C), (1, T)],
                        ),
                    )
                wf = wfp.tile([LW + 1, T], f32)
                wf_t[b] = wf
                # row LW must be 1.0 (bias trick); engines can only start at
                # partition 0/32/64/96: rows 96..100 are overwritten by the
                # window reduce below
                nc.gpsimd.memset(wf[96 : LW + 1, :], 1.0)

            def stageBC_half(b, h):
                # rank-1 PE broadcast of the base row (permuted row 0) into
                # PSUM, subtract in-place, |.|-reduce over c, mask only the
                # edge tiles, then matmul+relu+store the half's 4 t-tiles
                sh, wf = sh_t[b], wf_t[b]
                shv3 = sh[:].rearrange("p (c t) -> p c t", c=C)
                ba = pbap.tile([LW, C * TH], f32)
                for c in range(C):
                    nc.tensor.matmul(
                        ba[:, c * TH : (c + 1) * TH],
                        ones_sb[:],
                        shv3[0:1, c, h * TH : (h + 1) * TH],
                    )
                bav = ba[:].rearrange("p (c t) -> p c t", c=C)
                nc.vector.tensor_sub(
                    shv3[:, :, h * TH : (h + 1) * TH],
                    shv3[:, :, h * TH : (h + 1) * TH],
                    bav,
                )
                shv_tc = sh[:].rearrange("p (c t) -> p t c", c=C)
                nc.vector.tensor_reduce(
                    wf[0:LW, h * TH : (h + 1) * TH],
                    shv_tc[:, h * TH : (h + 1) * TH, :],
                    axis=mybir.AxisListType.X,
                    op=mybir.AluOpType.add,
                    apply_absolute_value=True,
                )
                # out-of-range wings only exist for t<50 and t>=974
                if h == 0:
                    nc.vector.tensor_mul(
                        wf[0:LW, 0:128], wf[0:LW, 0:128], maskw_sb[:, 0:128]
                    )
                else:
                    nc.vector.tensor_mul(
                        wf[0:LW, T - 128 : T],
                        wf[0:LW, T - 128 : T],
                        maskw_sb[:, T - 128 : T],
                    )
                for j in range(h * (FT // 2), (h + 1) * (FT // 2)):
                    po = pso.tile([128, OD], f32)
                    nc.tensor.matmul(po[:], wf[:, bass.ts(j, 128)], fcwb_sb[:])
                    osb = outsp.tile([128, OD], f32)
                    nc.vector.tensor_scalar_max(osb[:], po[:], 0.0)
                    # batch 0 outputs go mid-stream on the idle gpsimd ring;
                    # batch 1 outputs at the tail split across sync+scalar
                    # (both idle by then) to halve issue time
                    if b == 0:
                        eng = nc.gpsimd
                    else:
                        eng = nc.sync if j % 2 == 0 else nc.scalar
                    eng.dma_start(
                        y[b * T + j * 128 : b * T + (j + 1) * 128, :], osb[:]
                    )

            # -------- emission schedule --------
            # b0's stage B/C is woven between b1's tile reduces so its
            # windowed distances + matmuls overlap the second half of the
            # x-stream without stalling any engine's FIFO.
            for b in range(BPC):
                sums_t[b] = sumsp.tile(
                    [128, C * FT], f32, name="sums", tag="sums"
                )
            for i in range(FT):
                stageA_tile(0, i)
            stageA_finish(0)
            stageB_gather(0, [nc.scalar])
            for i in range(0, 4):
                stageA_tile(1, i)
            stageBC_half(0, 0)
            stageBC_half(0, 1)
            for i in range(4, FT):
                stageA_tile(1, i)
            stageA_finish(1)
            stageB_gather(1, [nc.scalar, nc.sync])
            stageBC_half(1, 0)
            stageBC_half(1, 1)

    nc.compile()
    return nc


def get_nc():
    if "nc" not in _CACHE:
        _CACHE["nc"] = _build_program()
    return _CACHE["nc"]


def make_host_inputs(x, fc_w, fc_b):
    """Per-core input maps from the full problem inputs."""
    x = np.ascontiguousarray(x, dtype=np.float32).reshape(B, T, ROW)
    wT = fc_w.T.astype(np.float32)[PERM]          # window-row permutation
    fcwb = np.concatenate([wT, fc_b[None, :].astype(np.float32)], axis=0)
    fcwb = np.ascontiguousarray(fcwb)
    u = np.arange(T)[None, :] + np.arange(LW)[:, None] - PAD
    import ml_dtypes

    maskw = ((u >= 0) & (u < T)).astype(ml_dtypes.bfloat16)[PERM]
    maskw = np.ascontiguousarray(maskw)
    ident = np.eye(128, dtype=np.float32)
    in_maps = []
    for ci in range(N_CORES):
        shard = np.ascontiguousarray(
            x[ci * BPC : (ci + 1) * BPC].reshape(BPC * T, ROW)
        )
        in_maps.append(
            {"xs": shard, "fcwb": fcwb, "maskw": maskw, "ident": ident}
        )
    return in_maps


def kernel(x, fc_w, fc_b):
    from concourse.bass_utils import run_bass_kernel_spmd

    nc = get_nc()
    in_maps = make_host_inputs(x, fc_w, fc_b)
    res = run_bass_kernel_spmd(nc, in_maps, list(range(N_CORES)))
    outs = [r["y"].reshape(BPC, T, OD) for r in res.results]
    return np.concatenate(outs, axis=0).astype(np.float32)
